# revision 14
# baseline (speedup 1.0000x reference)
"""Trainium2 Bass kernel for nn_CirLinear (soft-NAS mixture of block-circulant
projections of a linear layer's weight, then y = x @ W_mix^T + bias).

v2 — bf16 + DMA-transpose rewrite.

Sharding: 2-way on tokens x 4-way on out_features (core c: token-half c//4,
out-quarter c%4). Each core gets 4096 tokens and 1024 out-features.

Host precomputes softmax(alphas) and the 256x256 block-mixing matrix M in
float64 (tiny math), ships M as bf16; x and weight are converted to bf16 on
host, with x pre-chunked k-major so the device can load x^T tiles via the
xbar DMA-transpose at full contiguous bandwidth.

Per-core device algorithm (all matmul operands bf16, PSUM accumulation fp32):
  1. x^T tiles [128 k, 4096 tok] via 16 DMA-transposes (no PE involvement).
  2. W_mix construction: funny-DMA W into block-packed layout, DVE riffle,
     PE-transpose to block-vectorized form, one 256-deep M matmul, transpose
     back, funny-DMA to a natural-layout DRAM scratch.
  3. W_mix^T tiles [128 k, 1024 o] via DMA-transpose reads of the scratch.
  4. GEMM: per 128-token tile, 16 matmuls (x^T chunk stationary, W_mix^T
     moving, N=512 per PSUM bank), fused bias-add on DVE, DMA out.
"""

import sys

import numpy as np

if "/opt/trn_rl_repo" not in sys.path:
    sys.path.insert(0, "/opt/trn_rl_repo")

import ml_dtypes

import concourse.bass as bass
import concourse.mybir as mybir
from concourse.tile import TileContext
from concourse.bass_utils import run_bass_kernel_spmd

F32 = mybir.dt.float32
BF16 = mybir.dt.bfloat16
BF16_NP = np.dtype(ml_dtypes.bfloat16)

IN_F = 1024
OUT_F = 4096
TOK = 16 * 512  # 8192 tokens
NCORES = 8
T_SHARD = 2  # token shards
O_SHARD = 4  # out-feature shards
TOKS = TOK // T_SHARD  # 4096 tokens per core
OSH = OUT_F // O_SHARD  # 1024 out-features per core
NQG = 2  # q-groups of 512 weight rows each (OSH = 1024)
NTILES = TOKS // 128  # 32 token tiles
KCH = IN_F // 128  # 8 contraction chunks
SEARCH_SPACE = [1, 2, 4, 8, 16]

_MAX_WAITS = 1


class _TC(TileContext):
    """Unmodified TileContext; kept as a hook point."""


def _split_excess_waits(nc: bass.Bass, max_waits: int = 1) -> None:
    """Move excess per-instruction sem-waits onto same-engine nops.

    The installed walrus rejects instructions carrying more than one
    sync-wait ("Too many sync wait commands"), but Tile freely attaches
    several.  Splitting them across nops placed immediately before the
    instruction on the same engine stream is semantically identical.
    """
    for fn in nc.m.functions:
        for bb in fn.blocks:
            out = []
            for inst in bb.instructions:
                si = inst.sync_info
                if si is not None and si.on_wait and len(si.on_wait) > max_waits:
                    waits = list(si.on_wait)
                    extra, keep = waits[:-max_waits], waits[-max_waits:]
                    for i in range(0, len(extra), max_waits):
                        nop = mybir.InstNoOp(
                            name=nc.get_next_instruction_name(), ins=[], outs=[]
                        )
                        nop.engine = inst.engine
                        nop.bass_nofuse = True
                        nop.sync_info = mybir.SyncInfo(
                            on_wait=extra[i : i + max_waits], on_update=[]
                        )
                        nc.register_instruction(nop, overwrite=True)
                        out.append(nop)
                    si.on_wait = keep
                out.append(inst)
            bb.instructions[:] = out


def make_masks() -> np.ndarray:
    """Constant [5, 256, 256] mixing masks (already scaled by 1/bs).

    M[(k,j),(k',j')] for block size bs is 1/bs iff k,k' share a bs-sub-block,
    j,j' share a bs-sub-block, and (k-j)+(k'-j') == 0 (mod bs).  The bs=1 term
    is the identity (original-weight passthrough).
    """
    r = np.arange(16)
    kk, jj, kk2, jj2 = np.meshgrid(r, r, r, r, indexing="ij")
    out = np.zeros((5, 256, 256), dtype=np.float64)
    for i, bs in enumerate(SEARCH_SPACE):
        cond = (
            (kk // bs == kk2 // bs)
            & (jj // bs == jj2 // bs)
            & (((kk - jj) + (kk2 - jj2)) % bs == 0)
        )
        out[i] = cond.reshape(256, 256).astype(np.float64) / bs
    return out


def _funny_dram_ap(handle, g: int, k: int) -> bass.AP:
    """AP over the (g,k)-slice of a [1024, 1024] DRAM tensor matching the
    block-packed SBUF tile [128 part = q*4 + p_hi, free = p_lo*16 + j] where
    the DRAM element at (512g + 16q + k, 256*p_hi + 16*p_lo + j) maps to
    (part, free).  One DMA per (g,k) keeps APs within the 3-dim limit."""
    return bass.AP(
        handle, g * 512 * 1024 + k * 1024, [[16384, 32], [256, 4], [1, 256]]
    )


def build_nc() -> bass.Bass:
    nc = bass.Bass()

    # x pre-chunked on host: xt[kc, t, i] = x_bf16[t, kc*128 + i]
    xt_d = nc.dram_tensor("xt", [KCH, TOKS, 128], BF16, kind="ExternalInput")
    w_d = nc.dram_tensor("w", [OSH, IN_F], BF16, kind="ExternalInput")
    mc_d = nc.dram_tensor("mc", [2, 128, 256], BF16, kind="ExternalInput")
    b_d = nc.dram_tensor("bias", [1, OSH], F32, kind="ExternalInput")
    ident_d = nc.dram_tensor("ident", [128, 128], BF16, kind="ExternalInput")
    y_d = nc.dram_tensor("y", [TOKS, OSH], F32, kind="ExternalOutput")

    with _TC(nc) as tc:
        with tc.tile_pool(name="persist", bufs=1) as persist:
            ident = persist.tile([128, 128], BF16, tag="ident")
            nc.sync.dma_start(ident[:, :], ident_d[:, :])
            mc = [
                persist.tile([128, 256], BF16, tag=f"mc{h}", name=f"mc{h}")
                for h in range(2)
            ]
            for h in range(2):
                nc.sync.dma_start(mc[h][:, :], mc_d[h, :, :])
            bias_f32 = persist.tile([1, OSH], F32, tag="bias_f32")
            nc.sync.dma_start(bias_f32[:, :], b_d[:, :])
            bias_bf = persist.tile([1, OSH], BF16, tag="bias_bf")
            nc.vector.tensor_copy(bias_bf[:, :], bias_f32[:, :])
            ones = persist.tile([1, 128], BF16, tag="ones")
            nc.vector.memset(ones[:, :], 1.0)
            bias128 = persist.tile([128, OSH], F32, tag="bias128")
            # single W_mix^T tile, kc-major: wmt[:, kc*OSH + o] = W_mix^T
            # tile for contraction chunk kc, out-feature o
            wmt = persist.tile([128, KCH * OSH], BF16, tag="wmt")
            # view used by the construction epilogue:
            # free = ph(4), plh(2), g(2), q(32), kg(4), ki(4)
            #      -> col = (2*ph+plh)*1024 + 16*(32g+q) + (4*kg+ki)
            wmt_r = wmt[:, :].rearrange(
                "p (ph plh hg q kg ki) -> p ph plh hg q kg ki",
                ph=4, plh=2, hg=2, q=32, kg=4, ki=4,
            )
            xT = [
                persist.tile([128, TOKS], BF16, tag=f"xT{kc}", name=f"xT{kc}")
                for kc in range(KCH)
            ]

            # ---- W_mix construction ----
            with (
                tc.tile_pool(name="wbuild", bufs=1) as wp,
                tc.tile_pool(name="pst", bufs=3, space="PSUM") as pst,
            ):
                # one-time bias broadcast to 128 partitions (K=1 matmul)
                for h in range(2):
                    pb = pst.tile([128, 512], F32, tag="pc", name="pb")
                    nc.tensor.matmul(
                        pb[:, :],
                        ones[:, :],
                        bias_bf[:, h * 512 : (h + 1) * 512],
                        start=True,
                        stop=True,
                    )
                    nc.vector.tensor_copy(bias128[:, h * 512 : (h + 1) * 512], pb[:, :])

                # ---- stage-major over the two 512-row groups so engines
                # pipeline: while PE transposes group 0, DMA/DVE prepare
                # group 1.  Funny loads go on two different queues.
                wfl, wrl, bvl, bml, wmfl = [], [], [], [], []
                for g in range(NQG):
                    wf = wp.tile([128, 4096], BF16, tag=f"wf{g}", name=f"wf{g}")
                    wf4 = wf[:, :].rearrange("p (k pl j) -> p k pl j", k=16, pl=16, j=16)
                    for k in range(16):
                        # g0 alternates the two HWDGE rings; g1 uses SWDGE
                        eng = nc.gpsimd if g else (nc.scalar if k % 2 == 0 else nc.sync)
                        eng.dma_start(wf4[:, k, :, :], _funny_dram_ap(w_d, g, k))
                    wfl.append(wf4)

                # ---- x^T via xbar DMA-transpose (fully contiguous source),
                # issued after the g0 funny loads on the sync ring.
                # Quarter-major so early GEMM tiles unblock first.
                for quarter in range(4):
                    for kc in range(KCH):
                        nc.sync.dma_start(
                            xT[kc][:, quarter * 1024 : (quarter + 1) * 1024],
                            xt_d[kc, quarter * 1024 : (quarter + 1) * 1024, :],
                            transpose=True,
                        )
                for g in range(NQG):
                    # reorder free dims (k, p_lo, j) -> (p_lo, k, j) so each
                    # (k-half, j) transpose input is one contiguous 128-slice;
                    # halves on DVE and ACT in parallel
                    wr = wp.tile([128, 4096], BF16, tag=f"wr{g}", name=f"wr{g}")
                    wr4 = wr[:, :].rearrange("p (pl k j) -> p pl k j", pl=16, k=16, j=16)
                    nc.vector.tensor_copy(
                        wr4[:, 0:8, :, :],
                        wfl[g][:, :, 0:8, :].rearrange("p k pl j -> p pl k j"),
                    )
                    nc.scalar.copy(
                        wr4[:, 8:16, :, :],
                        wfl[g][:, :, 8:16, :].rearrange("p k pl j -> p pl k j"),
                    )
                    wrl.append(wr)
                for g in range(NQG):
                    bv = [
                        wp.tile([128, 2048], BF16, tag=f"bv{g}{h}", name=f"bv{g}{h}")
                        for h in range(2)
                    ]
                    for h in range(2):
                        for plg in range(4):
                            tp = pst.tile([128, 512], BF16, tag="pc", name="tp")
                            for pi in range(4):
                                pl = plg * 4 + pi
                                nc.tensor.transpose(
                                    tp[:, pi * 128 : (pi + 1) * 128],
                                    wrl[g][:, pl * 256 + h * 128 : pl * 256 + (h + 1) * 128],
                                    ident[:, :],
                                )
                            if (h * 4 + plg) % 2 == 0:
                                nc.vector.tensor_copy(
                                    bv[h][:, plg * 512 : (plg + 1) * 512], tp[:, :]
                                )
                            else:
                                nc.scalar.copy(
                                    bv[h][:, plg * 512 : (plg + 1) * 512], tp[:, :]
                                )
                    bvl.append(bv)
                for g in range(NQG):
                    bm = [
                        wp.tile([128, 2048], BF16, tag=f"bm{g}{hr}", name=f"bm{g}{hr}")
                        for hr in range(2)
                    ]
                    for hr in range(2):
                        for nch in range(4):
                            mp = pst.tile([128, 512], F32, tag="pc", name="mp")
                            for hc in range(2):
                                nc.tensor.matmul(
                                    mp[:, :],
                                    mc[hc][:, hr * 128 : (hr + 1) * 128],
                                    bvl[g][hc][:, nch * 512 : (nch + 1) * 512],
                                    start=(hc == 0),
                                    stop=(hc == 1),
                                )
                            nc.scalar.copy(
                                bm[hr][:, nch * 512 : (nch + 1) * 512], mp[:, :]
                            )
                    bml.append(bm)
                for g in range(NQG):
                    wmf = wp.tile([128, 4096], BF16, tag=f"wmf{g}", name=f"wmf{g}")
                    wmf4 = wmf[:, :].rearrange(
                        "p (k pl j) -> p k pl j", k=16, pl=16, j=16
                    )
                    for hr in range(2):
                        for plg in range(4):
                            tb = pst.tile([128, 512], BF16, tag="pc", name="tb")
                            for pi in range(4):
                                pl = plg * 4 + pi
                                nc.tensor.transpose(
                                    tb[:, pi * 128 : (pi + 1) * 128],
                                    bml[g][hr][:, pl * 128 : (pl + 1) * 128],
                                    ident[:, :],
                                )
                            src = tb[:, :].rearrange(
                                "p (pl k j) -> p k pl j", pl=4, k=8, j=16
                            )
                            nc.vector.tensor_copy(
                                wmf4[
                                    :, hr * 8 : (hr + 1) * 8, plg * 4 : (plg + 1) * 4, :
                                ],
                                src,
                            )
                    wmfl.append(wmf)
                for g in range(NQG):
                    # packed W_mix -> W_mix^T directly on-chip: for each
                    # (k, pl-half), transpose the contiguous (pl8, j) slice so
                    # the in_feature index lands on partitions, then riffle the
                    # (q, p_hi) free dim into kc-major wmt columns on DVE.
                    for plh in range(2):
                        for kg in range(4):
                            tps = pst.tile([128, 512], BF16, tag="pc", name="tps")
                            for ki in range(4):
                                k = kg * 4 + ki
                                nc.tensor.transpose(
                                    tps[:, ki * 128 : (ki + 1) * 128],
                                    wmfl[g][:, k * 256 + plh * 128 : k * 256 + (plh + 1) * 128],
                                    ident[:, :],
                                )
                            src = tps[:, :].rearrange(
                                "p (ki q ph) -> p ph q ki", ki=4, q=32, ph=4
                            )
                            nc.vector.tensor_copy(
                                wmt_r[:, :, plh, g, :, kg, :], src
                            )

            # ---- main GEMM over token tiles ----
            with (
                tc.tile_pool(name="yout", bufs=4) as yout,
                tc.tile_pool(name="psy", bufs=3, space="PSUM") as psy,
            ):
                for tt in range(NTILES):
                    yps = psy.tile([128, OSH], F32, tag="yps")
                    for h in range(2):
                        for kc in range(KCH):
                            base = kc * OSH + h * 512
                            nc.tensor.matmul(
                                yps[:, h * 512 : (h + 1) * 512],
                                xT[kc][:, tt * 128 : (tt + 1) * 128],
                                wmt[:, base : base + 512],
                                start=(kc == 0),
                                stop=(kc == KCH - 1),
                            )
                    ysb = yout.tile([128, OSH], F32, tag="ysb")
                    for h in range(2):
                        # per-half eviction: h0 copies out while h1 matmuls run
                        nc.vector.scalar_tensor_tensor(
                            ysb[:, h * 512 : (h + 1) * 512],
                            yps[:, h * 512 : (h + 1) * 512],
                            1.0,
                            bias128[:, h * 512 : (h + 1) * 512],
                            mybir.AluOpType.mult,
                            mybir.AluOpType.add,
                        )
                    nc.gpsimd.dma_start(y_d[tt * 128 : (tt + 1) * 128, :], ysb[:, :])

    _split_excess_waits(nc)
    return nc


_NC_CACHE: dict = {}


def _get_nc() -> bass.Bass:
    if "nc" not in _NC_CACHE:
        _NC_CACHE["nc"] = build_nc()
    return _NC_CACHE["nc"]


def make_in_maps(x, weight, alphas, bias):
    x2 = np.asarray(x, dtype=np.float32).reshape(TOK, IN_F)
    x_bf = x2.astype(BF16_NP)
    weight_bf = np.asarray(weight, dtype=np.float32).astype(BF16_NP)
    bias = np.asarray(bias, dtype=np.float32)

    # host-side softmax + mixing matrix (float64; rounds once to bf16)
    al = np.asarray(alphas, dtype=np.float64).reshape(5)
    a = np.exp(al - al.max())
    a = a / a.sum()
    M = np.einsum("i,iab->ab", a, make_masks())  # [256, 256], symmetric
    mc = np.ascontiguousarray(M.reshape(2, 128, 256)).astype(BF16_NP)

    ident = np.eye(128, dtype=np.float32).astype(BF16_NP)

    # per-token-half pre-chunked x^T sources: [KCH, TOKS, 128]
    xt_halves = []
    for th in range(T_SHARD):
        xh = x_bf[th * TOKS : (th + 1) * TOKS]  # [TOKS, 1024]
        xt = np.ascontiguousarray(
            xh.reshape(TOKS, KCH, 128).transpose(1, 0, 2)
        )  # [KCH, TOKS, 128]
        xt_halves.append(xt)

    in_maps = []
    for c in range(NCORES):
        th, oq = c // O_SHARD, c % O_SHARD
        in_maps.append(
            {
                "xt": xt_halves[th],
                "w": np.ascontiguousarray(weight_bf[oq * OSH : (oq + 1) * OSH]),
                "mc": mc,
                "bias": np.ascontiguousarray(
                    bias[oq * OSH : (oq + 1) * OSH]
                ).reshape(1, OSH),
                "ident": ident,
            }
        )
    return in_maps


def run(x, weight, alphas, bias, trace=False, **rkw):
    nc = _get_nc()
    in_maps = make_in_maps(x, weight, alphas, bias)
    res = run_bass_kernel_spmd(nc, in_maps, list(range(NCORES)), trace=trace, **rkw)
    y = np.empty((TOK, OUT_F), dtype=np.float32)
    for c in range(NCORES):
        th, oq = c // O_SHARD, c % O_SHARD
        y[th * TOKS : (th + 1) * TOKS, oq * OSH : (oq + 1) * OSH] = res.results[c]["y"]
    return y.reshape(16, 512, OUT_F), res


def kernel(x, weight, alphas, bias):
    y, _ = run(x, weight, alphas, bias)
    return y.astype(np.float32)


if __name__ == "__main__":
    rng = np.random.default_rng(0)
    x = rng.standard_normal((16, 512, IN_F), dtype=np.float32)
    w = (rng.standard_normal((OUT_F, IN_F)) * 0.02).astype(np.float32)
    a = rng.standard_normal(5).astype(np.float32)
    b = (rng.standard_normal(OUT_F) * 0.02).astype(np.float32)
    y = kernel(x=x, weight=w, alphas=a, bias=b)
    print("y", y.shape, y.dtype, float(np.abs(y).max()))
